# revision 15
# baseline (speedup 1.0000x reference)
"""GemmaAttention (B=2, S=2048, D=2048, H=8, KV=1, HD=256) on 8 trn2 NeuronCores.

Sharding: DP=2 over batch x TP=4 over head-pairs. Core c handles batch c//4 and
heads {2*(c%4), 2*(c%4)+1}. Each core computes its partial o_proj output
(row-parallel Wo); the host sums the 4 partials per batch (the all-reduce is
folded into the host-side unshard).

All PE matmuls run in bf16 (error budget allows it): LDWEIGHTS is 1 cycle/row
instead of fp32r's ~4, and PE power/p-state behaves better. PSUM tiles are
always a full bank, so the kernel is structured around at most 8 live psum
tiles with single-group accumulation chains to keep the PE gap-free (the PE
only reaches its 2.4 GHz p-state after ~3us of continuous execution).

Dataflow per core:
  phase A (projections, full hT resident in SBUF as bf16):
    per s-block of 512 and output pair: psum <- sum_c W[:,c,pair].T @ hT[c,blk]
    RoPE applied in the psum->SBUF drain on DVE, output bf16 (QT/KT).
    V computed directly in [s, dv] layout: psum <- hT_chunk.T @ Wv, drained on
    the scalar engine to bf16 (VN).
  phase C (attention + interleaved o_proj):
    scoresT[k,q] = KT_chunk.T @ QT per head, exp on ACT (scale 1/16 folded),
    causal staircase as a bf16 multiplicative mask, denominators accumulated
    on DVE; outT[dv,q] += V_chunk.T @ expT; normalize with
    reciprocal_approx_fast; out_partial = outTn_chunk.T @ Wo -> DRAM.
"""

import numpy as np
import ml_dtypes

import concourse.bass as bass
import concourse.tile as tile
import concourse.mybir as mybir
from concourse import bacc
from concourse.bass_utils import run_bass_kernel_spmd
from concourse._compat import with_exitstack  # noqa: F401

P = 128
B, S, D = 2, 2048, 2048
H, KV, HD = 8, 1, 256
ROPE_BASE = 10000.0

HEADS_PER_CORE = 2
DQ = HEADS_PER_CORE * HD          # 512 q-dims per core
DCH = D // P                      # 16 contraction chunks
SBLK = 512                        # s-tile for projection rhs / q-tile
NSBLK = S // SBLK                 # 4
NKC = S // P                      # 16 key chunks
NQCH = DQ // P                    # 4 QT partition chunks
NKCH = HD // P                    # 2 KT partition chunks

F32 = mybir.dt.float32
F32R = mybir.dt.float32r
BF16 = mybir.dt.bfloat16
EXP = mybir.ActivationFunctionType.Exp

# exec time of the last traced run (set by run_spmd when tracing)
LAST_EXEC_TIME_NS = None

_BUILD_CACHE = {}


def _build(causal: bool):
    nc = bacc.Bacc()

    hT = nc.declare_dram_parameter("hT", [D, S], BF16, isOutput=False)
    wq = nc.declare_dram_parameter("wq", [D, DQ], BF16, isOutput=False)
    wk = nc.declare_dram_parameter("wk", [D, HD], BF16, isOutput=False)
    wv = nc.declare_dram_parameter("wv", [D, HD], BF16, isOutput=False)
    wo = nc.declare_dram_parameter("wo", [DQ, D], BF16, isOutput=False)
    cosT = nc.declare_dram_parameter("cosT", [HD, S], F32, isOutput=False)
    sinT = nc.declare_dram_parameter("sinT", [HD, S], F32, isOutput=False)
    ones = nc.declare_dram_parameter("ones", [P, P], F32R, isOutput=False)
    onesb = nc.declare_dram_parameter("onesb", [P, P], BF16, isOutput=False)
    if causal:
        stair = nc.declare_dram_parameter("stair", [P, 2 * SBLK], BF16, isOutput=False)
    else:
        maskT = nc.declare_dram_parameter("emaskT", [S, S], BF16, isOutput=False)
    outp = nc.declare_dram_parameter("out_partial", [S, D], BF16, isOutput=True)

    from contextlib import ExitStack
    from collections import deque
    with tile.TileContext(nc) as tc, ExitStack() as ctx:
        # persistent pools
        pq = ctx.enter_context(tc.tile_pool(name="pq", bufs=1))
        QT = pq.tile([P, NQCH, S], BF16, name="QT")
        KT = pq.tile([P, NKCH, S], BF16, name="KT")
        VN = pq.tile([P, NKC, HD], BF16, name="VN")
        ONES = pq.tile([P, P], F32R, name="ONES")
        ONESB = pq.tile([P, P], BF16, name="ONESB")
        ONEC = ONES[:, 0:1]
        ONERB = ONESB[0:1, :]
        if causal:
            STAIR = pq.tile([P, 2 * SBLK], BF16, name="STAIR")

        # ---- phase A: projections + RoPE --------------------------------
        with tc.tile_pool(name="pht", bufs=1) as pht, \
             tc.tile_pool(name="pw", bufs=1) as pw, \
             tc.tile_pool(name="pcs", bufs=1) as pcs, \
             tc.tile_pool(name="ptmp", bufs=8) as ptmp, \
             tc.tile_pool(name="pjp", bufs=8, space="PSUM") as pp:
            HT = pht.tile([P, DCH, S], BF16, name="HT")
            WQ = pw.tile([P, DCH, DQ], BF16, name="WQ")
            WK = pw.tile([P, DCH, HD], BF16, name="WK")
            WV = pw.tile([P, DCH, HD], BF16, name="WV")
            COS = pcs.tile([P, NKCH, S], F32, name="COS")
            SIN = pcs.tile([P, NKCH, S], F32, name="SIN")

            # DMA order matches consumption. Multi-chunk 3D transfers keep the
            # sync queue from becoming issue-bound (~620ns per descriptor).
            nc.sync.dma_start(out=ONES, in_=ones[:, :])
            nc.sync.dma_start(out=ONESB, in_=onesb[:, :])
            if causal:
                nc.sync.dma_start(out=STAIR, in_=stair[:, :])

            def chunked3d(out_tile, dram, c0, c1, csl):
                # out_tile[:, c0:c1, csl] <- dram rows [c0*P, c1*P), cols csl
                nc.sync.dma_start(
                    out=out_tile[:, c0:c1, csl],
                    in_=dram.rearrange("(c p) n -> p c n", p=P)[:, c0:c1, csl])

            full = slice(0, None)
            # sb0: WQ and hT in 4-chunk slabs, interleaved for fast start
            for g in range(4):
                chunked3d(WQ, wq, 4 * g, 4 * g + 4, full)
                chunked3d(HT, hT, 4 * g, 4 * g + 4, slice(0, SBLK))
            chunked3d(WK, wk, 0, DCH, full)
            chunked3d(WV, wv, 0, DCH, full)
            chunked3d(COS, cosT, 0, NKCH, full)
            chunked3d(SIN, sinT, 0, NKCH, full)
            for sb in range(1, NSBLK):
                chunked3d(HT, hT, 0, DCH, slice(sb * SBLK, (sb + 1) * SBLK))

            def rope_pair(p0, p1, out0, out1):
                # out0 = p0*cos0 - p1*sin0 ; out1 = p1*cos1 + p0*sin1
                c0 = COS[:, 0, ssl]; c1 = COS[:, 1, ssl]
                s0 = SIN[:, 0, ssl]; s1 = SIN[:, 1, ssl]
                t1 = ptmp.tile([P, SBLK], F32, name="t")
                t2 = ptmp.tile([P, SBLK], F32, name="t")
                nc.vector.tensor_mul(t1, p0, c0)
                nc.vector.tensor_mul(t2, p1, s0)
                nc.vector.tensor_sub(out0, t1, t2)
                t3 = ptmp.tile([P, SBLK], F32, name="t")
                t4 = ptmp.tile([P, SBLK], F32, name="t")
                nc.vector.tensor_mul(t3, p1, c1)
                nc.vector.tensor_mul(t4, p0, s1)
                nc.vector.tensor_add(out1, t3, t4)

            for sb in range(NSBLK):
                ssl = slice(sb * SBLK, (sb + 1) * SBLK)
                # Q pairs (per head), then K pair: 16-matmul psum groups
                for h in range(HEADS_PER_CORE):
                    p0 = pp.tile([P, SBLK], F32, name="pp")
                    p1 = pp.tile([P, SBLK], F32, name="pp")
                    for c in range(DCH):
                        nc.tensor.matmul(p0, lhsT=WQ[:, c, 2 * h * P:(2 * h + 1) * P],
                                         rhs=HT[:, c, ssl],
                                         start=(c == 0), stop=(c == DCH - 1))
                        nc.tensor.matmul(p1, lhsT=WQ[:, c, (2 * h + 1) * P:(2 * h + 2) * P],
                                         rhs=HT[:, c, ssl],
                                         start=(c == 0), stop=(c == DCH - 1))
                    rope_pair(p0, p1, QT[:, 2 * h, ssl], QT[:, 2 * h + 1, ssl])
                p0 = pp.tile([P, SBLK], F32, name="pp")
                p1 = pp.tile([P, SBLK], F32, name="pp")
                for c in range(DCH):
                    nc.tensor.matmul(p0, lhsT=WK[:, c, 0:P], rhs=HT[:, c, ssl],
                                     start=(c == 0), stop=(c == DCH - 1))
                    nc.tensor.matmul(p1, lhsT=WK[:, c, P:2 * P], rhs=HT[:, c, ssl],
                                     start=(c == 0), stop=(c == DCH - 1))
                rope_pair(p0, p1, KT[:, 0, ssl], KT[:, 1, ssl])
                # V directly in [s, dv] layout (lhsT = hT chunk slice)
                for si in range(SBLK // P):
                    sv = pp.tile([P, SBLK], F32, name="pp")[:, 0:HD]
                    soff = sb * SBLK + si * P
                    for c in range(DCH):
                        nc.tensor.matmul(sv, lhsT=HT[:, c, soff:soff + P],
                                         rhs=WV[:, c, :],
                                         start=(c == 0), stop=(c == DCH - 1))
                    nc.scalar.copy(VN[:, sb * (SBLK // P) + si, :], sv)

        # ---- late persistent: o_proj weights + normalized outT ----------
        patt = ctx.enter_context(tc.tile_pool(name="patt", bufs=1))
        WO = patt.tile([P, NQCH, D], BF16, name="WO")
        nc.sync.dma_start(out=WO, in_=wo.rearrange("(c p) n -> p c n", p=P))
        OUTN = patt.tile([P, NQCH, S], BF16, name="OUTN")

        # ---- phase C: attention + interleaved o_proj --------------------
        with tc.tile_pool(name="pexp", bufs=8) as pexp, \
             tc.tile_pool(name="pacc", bufs=4) as pacc, \
             tc.tile_pool(name="pou", bufs=8) as pou, \
             tc.tile_pool(name="pmisc", bufs=6) as pmisc, \
             tc.tile_pool(name="pmask", bufs=2) as pmask, \
             tc.tile_pool(name="pfin", bufs=3) as pfin, \
             tc.tile_pool(name="ps_s", bufs=4, space="PSUM") as ps_s, \
             tc.tile_pool(name="ps_o", bufs=2, space="PSUM") as ps_o, \
             tc.tile_pool(name="ps_f", bufs=2, space="PSUM") as ps_f:

            def emit_norm(pend):
                ou, acc, h, qb = pend
                qsl = slice(qb * SBLK, (qb + 1) * SBLK)
                pssum = ps_s.tile([P, SBLK], F32, name="ps")
                nc.tensor.matmul(pssum[0:1, :], lhsT=ONEC, rhs=acc)
                rsb = pmisc.tile([1, SBLK], F32, name="rsb")
                rsbb = pmisc.tile([1, SBLK], BF16, name="rsbb")
                with nc.allow_low_precision("approx reciprocal of softmax sums"):
                    nc.vector.reciprocal_approx_fast(out=rsb, in_=pssum[0:1, :])
                    nc.vector.tensor_copy(rsbb, rsb)
                psb = ps_f.tile([P, SBLK], F32, name="pf")
                nc.tensor.matmul(psb, lhsT=ONERB, rhs=rsbb)
                rbc = pmisc.tile([P, SBLK], F32R, name="rbc")
                nc.scalar.copy(rbc, psb)
                for dvc in range(2):
                    nc.vector.tensor_mul(OUTN[:, 2 * h + dvc, qsl], ou[dvc], rbc)

            def emit_oproj_quarter(qb):
                for st in range(4 * qb, 4 * qb + 4):
                    stsl = slice(st * P, (st + 1) * P)
                    for nb in range(NSBLK):
                        psf = ps_f.tile([P, SBLK], F32, name="pf")
                        for dvc in range(NQCH):
                            nc.tensor.matmul(psf, lhsT=OUTN[:, dvc, stsl],
                                             rhs=WO[:, dvc, nb * SBLK:(nb + 1) * SBLK],
                                             start=(dvc == 0), stop=(dvc == NQCH - 1))
                        # drain as bf16 (halves output DMA); alternate ACT/DVE
                        # so neither queue eats the full copy load (exp on ACT
                        # is latency-critical)
                        fsb = pfin.tile([P, SBLK], BF16, name="fsb")
                        if nb % 2 == 0:
                            nc.scalar.copy(fsb, psf)
                        else:
                            with nc.allow_low_precision("bf16 o_proj output"):
                                nc.vector.tensor_copy(fsb, psf)
                        nc.sync.dma_start(out=outp[stsl, nb * SBLK:(nb + 1) * SBLK],
                                          in_=fsb)

            pending = deque()
            for qb in range(NSBLK):
                qsl = slice(qb * SBLK, (qb + 1) * SBLK)
                klim = 4 * (qb + 1) if causal else NKC
                MT = None
                if not causal:
                    MT = pmask.tile([P, NKC, SBLK], BF16, name="mt")
                    nc.sync.dma_start(
                        out=MT,
                        in_=maskT.rearrange("(c p) n -> p c n", p=P)[:, :, qsl])
                for h in range(HEADS_PER_CORE):
                    pso = [ps_o.tile([P, SBLK], F32, name="po") for _ in range(2)]
                    acc = pacc.tile([P, SBLK], F32R, name="acc")
                    exs = {}

                    def emit_scores(kc):
                        pss = ps_s.tile([P, SBLK], F32, name="ps")
                        for c in range(NKCH):
                            nc.tensor.matmul(pss,
                                             lhsT=KT[:, c, kc * P:(kc + 1) * P],
                                             rhs=QT[:, 2 * h + c, qsl],
                                             start=(c == 0), stop=(c == NKCH - 1))
                        ex = pexp.tile([P, SBLK], BF16, name="ex")
                        nc.scalar.activation(ex, pss, EXP, scale=1.0 / 16.0)
                        if causal and kc >= 4 * qb:
                            delta = 128 * kc - 512 * qb
                            nc.vector.tensor_mul(ex, ex,
                                                 STAIR[:, 512 - delta:1024 - delta])
                        if not causal:
                            nc.vector.tensor_mul(ex, ex, MT[:, kc, :])
                        if kc == 0:
                            nc.vector.tensor_copy(acc, ex)
                        else:
                            nc.vector.tensor_add(acc, acc, ex)
                        exs[kc] = ex

                    def emit_av(kc):
                        ex = exs.pop(kc)
                        for dvc in range(2):
                            nc.tensor.matmul(pso[dvc],
                                             lhsT=VN[:, kc, dvc * P:(dvc + 1) * P],
                                             rhs=ex, start=(kc == 0),
                                             stop=(kc == klim - 1))

                    LOOKAHEAD = 4
                    for kc in range(klim):
                        emit_scores(kc)
                        if kc >= LOOKAHEAD:
                            emit_av(kc - LOOKAHEAD)
                    for kc in range(max(0, klim - LOOKAHEAD), klim):
                        emit_av(kc)
                    ou = [pou.tile([P, SBLK], BF16, name="ou") for _ in range(2)]
                    for dvc in range(2):
                        nc.vector.tensor_copy(ou[dvc], pso[dvc])
                    pending.append((ou, acc, h, qb))
                    if len(pending) > 2:
                        p = pending.popleft()
                        emit_norm(p)
                        if p[2] == 1:
                            emit_oproj_quarter(p[3])
            while pending:
                p = pending.popleft()
                emit_norm(p)
                if p[2] == 1:
                    emit_oproj_quarter(p[3])

    nc.finalize()
    return nc


def _get_nc(causal: bool):
    key = bool(causal)
    if key not in _BUILD_CACHE:
        _BUILD_CACHE[key] = _build(causal)
    return _BUILD_CACHE[key]


def _rope_tables(position_ids_b):
    # cosT/sinT: [HD, S] fp32, transposed layout for the [d, s] dataflow
    pos = np.asarray(position_ids_b, dtype=np.float64)
    inv = 1.0 / (ROPE_BASE ** (np.arange(0, HD, 2, dtype=np.float64) / HD))
    f = pos[:, None] * inv[None, :]            # [S, HD/2]
    emb = np.concatenate([f, f], axis=1)       # [S, HD]
    cosT = np.ascontiguousarray(np.cos(emb).T.astype(np.float32))
    sinT = np.ascontiguousarray(np.sin(emb).T.astype(np.float32))
    return cosT, sinT


def _is_causal(attention_mask):
    m = np.asarray(attention_mask)
    if m.shape != (B, 1, S, S):
        return False
    tri = np.tril(np.ones((S, S), dtype=bool))
    canon = np.where(tri, np.float32(0.0), np.float32(-1e9))
    return all(np.array_equal(m[b, 0], canon) for b in range(B))


_ONES_NP = np.ones((P, P), dtype=np.float32)
_ONES_BF = np.ones((P, P), dtype=ml_dtypes.bfloat16)


def _stair():
    # multiplicative staircase: stair[p, j] = 1 if (j - 512) >= p else 0
    j = np.arange(2 * SBLK)[None, :] - SBLK
    p = np.arange(P)[:, None]
    return np.where(j >= p, 1.0, 0.0).astype(ml_dtypes.bfloat16)


def _bf(x):
    return np.ascontiguousarray(np.asarray(x, dtype=np.float32).astype(ml_dtypes.bfloat16))


def kernel(hidden_state, attention_mask, position_ids, Wq, Wk, Wv, Wo,
           _trace=False, _tmpdir=None):
    global LAST_EXEC_TIME_NS
    hidden_state = np.asarray(hidden_state, dtype=np.float32)

    causal = _is_causal(attention_mask)
    nc = _get_nc(causal)

    stair = _stair() if causal else None
    wk_bf = _bf(Wk)
    wv_bf = _bf(Wv)
    per_batch = {}
    for b in range(B):
        hTb = _bf(hidden_state[b].T)                           # [D, S] bf16
        cosT, sinT = _rope_tables(position_ids[b])
        mb = None
        if not causal:
            mb = np.ascontiguousarray(
                np.exp(np.asarray(attention_mask, dtype=np.float64)[b, 0].T)
                .astype(ml_dtypes.bfloat16))
        per_batch[b] = (hTb, cosT, sinT, mb)

    in_maps = []
    for core in range(8):
        b = core // 4
        hp = core % 4
        hTb, cosT, sinT, mb = per_batch[b]
        im = {
            "hT": hTb,
            "ones": _ONES_NP,
            "onesb": _ONES_BF,
            "wq": _bf(Wq[:, hp * DQ:(hp + 1) * DQ]),
            "wk": wk_bf,
            "wv": wv_bf,
            "wo": _bf(Wo[hp * DQ:(hp + 1) * DQ, :]),
            "cosT": cosT,
            "sinT": sinT,
        }
        if causal:
            im["stair"] = stair
        else:
            im["emaskT"] = mb
        in_maps.append(im)

    res = run_bass_kernel_spmd(nc, in_maps, core_ids=list(range(8)),
                               trace=_trace, tmpdir=_tmpdir)
    LAST_EXEC_TIME_NS = res.exec_time_ns

    out = np.empty((B, S, D), dtype=np.float32)
    for b in range(B):
        acc = res.results[4 * b]["out_partial"].astype(np.float32).copy()
        for hp in range(1, 4):
            acc += res.results[4 * b + hp]["out_partial"]
        out[b] = acc
    return out


# revision 18
# speedup vs baseline: 1.1038x; 1.1038x over previous
"""GemmaAttention (B=2, S=2048, D=2048, H=8, KV=1, HD=256) on 8 trn2 NeuronCores.

Sharding: DP=2 over batch x TP=4 over head-pairs. Core c handles batch c//4 and
heads {2*(c%4), 2*(c%4)+1}. Each core computes its partial o_proj output
(row-parallel Wo); the host sums the 4 partials per batch (the all-reduce is
folded into the host-side unshard).

All PE matmuls run in bf16 (error budget allows it): LDWEIGHTS is 1 cycle/row
instead of fp32r's ~4, and PE power/p-state behaves better. PSUM tiles are
always a full bank, so the kernel is structured around at most 8 live psum
tiles with single-group accumulation chains to keep the PE gap-free (the PE
only reaches its 2.4 GHz p-state after ~3us of continuous execution).

Dataflow per core:
  phase A (projections, full hT resident in SBUF as bf16):
    per s-block of 512 and output pair: psum <- sum_c W[:,c,pair].T @ hT[c,blk]
    RoPE applied in the psum->SBUF drain on DVE, output bf16 (QT/KT).
    V computed directly in [s, dv] layout: psum <- hT_chunk.T @ Wv, drained on
    the scalar engine to bf16 (VN).
  phase C (attention + interleaved o_proj):
    scoresT[k,q] = KT_chunk.T @ QT per head, exp on ACT (scale 1/16 folded),
    causal staircase as a bf16 multiplicative mask, denominators accumulated
    on DVE; outT[dv,q] += V_chunk.T @ expT; normalize with
    reciprocal_approx_fast; out_partial = outTn_chunk.T @ Wo -> DRAM.
"""

import numpy as np
import ml_dtypes

import concourse.bass as bass
import concourse.tile as tile
import concourse.mybir as mybir
from concourse import bacc
from concourse.bass_utils import run_bass_kernel_spmd
from concourse._compat import with_exitstack  # noqa: F401

P = 128
B, S, D = 2, 2048, 2048
H, KV, HD = 8, 1, 256
ROPE_BASE = 10000.0

HEADS_PER_CORE = 2
DQ = HEADS_PER_CORE * HD          # 512 q-dims per core
DCH = D // P                      # 16 contraction chunks
SBLK = 512                        # s-tile for projection rhs / q-tile
NSBLK = S // SBLK                 # 4
NKC = S // P                      # 16 key chunks
NQCH = DQ // P                    # 4 QT partition chunks
NKCH = HD // P                    # 2 KT partition chunks

F32 = mybir.dt.float32
F32R = mybir.dt.float32r
BF16 = mybir.dt.bfloat16
EXP = mybir.ActivationFunctionType.Exp

# exec time of the last traced run (set by run_spmd when tracing)
LAST_EXEC_TIME_NS = None

_BUILD_CACHE = {}


def _build(causal: bool):
    nc = bacc.Bacc()

    hT = nc.declare_dram_parameter("hT", [D, S], BF16, isOutput=False)
    wq = nc.declare_dram_parameter("wq", [D, DQ], BF16, isOutput=False)
    wk = nc.declare_dram_parameter("wk", [D, HD], BF16, isOutput=False)
    wv = nc.declare_dram_parameter("wv", [D, HD], BF16, isOutput=False)
    wo = nc.declare_dram_parameter("wo", [DQ, D], BF16, isOutput=False)
    cosT = nc.declare_dram_parameter("cosT", [HD, S], F32, isOutput=False)
    sinT = nc.declare_dram_parameter("sinT", [HD, S], F32, isOutput=False)
    ones = nc.declare_dram_parameter("ones", [P, P], F32R, isOutput=False)
    onesb = nc.declare_dram_parameter("onesb", [P, P], BF16, isOutput=False)
    if causal:
        stair = nc.declare_dram_parameter("stair", [P, 2 * SBLK], BF16, isOutput=False)
    else:
        maskT = nc.declare_dram_parameter("emaskT", [S, S], BF16, isOutput=False)
    outp = nc.declare_dram_parameter("out_partial", [S, D], BF16, isOutput=True)

    from contextlib import ExitStack
    from collections import deque
    with tile.TileContext(nc) as tc, ExitStack() as ctx:
        # persistent pools
        pq = ctx.enter_context(tc.tile_pool(name="pq", bufs=1))
        QT = pq.tile([P, NQCH, S], BF16, name="QT")
        KT = pq.tile([P, NKCH, S], BF16, name="KT")
        VN = pq.tile([P, NKC, HD], BF16, name="VN")
        ONES = pq.tile([P, P], F32R, name="ONES")
        ONESB = pq.tile([P, P], BF16, name="ONESB")
        ONEC = ONES[:, 0:1]
        ONERB = ONESB[0:1, :]
        if causal:
            STAIR = pq.tile([P, 2 * SBLK], BF16, name="STAIR")

        # ---- phase A: projections + RoPE --------------------------------
        with tc.tile_pool(name="pht", bufs=1) as pht, \
             tc.tile_pool(name="pw", bufs=1) as pw, \
             tc.tile_pool(name="pcs", bufs=1) as pcs, \
             tc.tile_pool(name="ptmp", bufs=8) as ptmp, \
             tc.tile_pool(name="pjp", bufs=8, space="PSUM") as pp:
            HT = pht.tile([P, DCH, S], BF16, name="HT")
            WQ = pw.tile([P, DCH, DQ], BF16, name="WQ")
            WK = pw.tile([P, DCH, HD], BF16, name="WK")
            WV = pw.tile([P, DCH, HD], BF16, name="WV")
            COS = pcs.tile([P, NKCH, S], F32, name="COS")
            SIN = pcs.tile([P, NKCH, S], F32, name="SIN")

            # DMA order matches consumption. Multi-chunk 3D transfers keep the
            # sync queue from becoming issue-bound (~620ns per descriptor).
            nc.sync.dma_start(out=ONES, in_=ones[:, :])
            nc.sync.dma_start(out=ONESB, in_=onesb[:, :])
            if causal:
                nc.sync.dma_start(out=STAIR, in_=stair[:, :])

            def chunked3d(out_tile, dram, c0, c1, csl):
                # out_tile[:, c0:c1, csl] <- dram rows [c0*P, c1*P), cols csl
                nc.sync.dma_start(
                    out=out_tile[:, c0:c1, csl],
                    in_=dram.rearrange("(c p) n -> p c n", p=P)[:, c0:c1, csl])

            full = slice(0, None)
            # sb0: WQ and hT slabs interleaved, small first slab for fast start
            for c0, c1 in ((0, 2), (2, 6), (6, 11), (11, 16)):
                chunked3d(WQ, wq, c0, c1, full)
                chunked3d(HT, hT, c0, c1, slice(0, SBLK))
            chunked3d(WK, wk, 0, DCH, full)
            chunked3d(WV, wv, 0, DCH, full)
            chunked3d(COS, cosT, 0, NKCH, full)
            chunked3d(SIN, sinT, 0, NKCH, full)
            for sb in range(1, NSBLK):
                chunked3d(HT, hT, 0, DCH, slice(sb * SBLK, (sb + 1) * SBLK))

            def rope_pair(p0, p1, out0, out1):
                # out0 = p0*cos0 - p1*sin0 ; out1 = p1*cos1 + p0*sin1
                c0 = COS[:, 0, ssl]; c1 = COS[:, 1, ssl]
                s0 = SIN[:, 0, ssl]; s1 = SIN[:, 1, ssl]
                t1 = ptmp.tile([P, SBLK], F32, name="t")
                t2 = ptmp.tile([P, SBLK], F32, name="t")
                nc.vector.tensor_mul(t1, p0, c0)
                nc.vector.tensor_mul(t2, p1, s0)
                nc.vector.tensor_sub(out0, t1, t2)
                t3 = ptmp.tile([P, SBLK], F32, name="t")
                t4 = ptmp.tile([P, SBLK], F32, name="t")
                nc.vector.tensor_mul(t3, p1, c1)
                nc.vector.tensor_mul(t4, p0, s1)
                nc.vector.tensor_add(out1, t3, t4)

            for sb in range(NSBLK):
                ssl = slice(sb * SBLK, (sb + 1) * SBLK)
                # Q pairs (per head), then K pair: 16-matmul psum groups
                for h in range(HEADS_PER_CORE):
                    p0 = pp.tile([P, SBLK], F32, name="pp")
                    p1 = pp.tile([P, SBLK], F32, name="pp")
                    for c in range(DCH):
                        nc.tensor.matmul(p0, lhsT=WQ[:, c, 2 * h * P:(2 * h + 1) * P],
                                         rhs=HT[:, c, ssl],
                                         start=(c == 0), stop=(c == DCH - 1))
                        nc.tensor.matmul(p1, lhsT=WQ[:, c, (2 * h + 1) * P:(2 * h + 2) * P],
                                         rhs=HT[:, c, ssl],
                                         start=(c == 0), stop=(c == DCH - 1))
                    rope_pair(p0, p1, QT[:, 2 * h, ssl], QT[:, 2 * h + 1, ssl])
                p0 = pp.tile([P, SBLK], F32, name="pp")
                p1 = pp.tile([P, SBLK], F32, name="pp")
                for c in range(DCH):
                    nc.tensor.matmul(p0, lhsT=WK[:, c, 0:P], rhs=HT[:, c, ssl],
                                     start=(c == 0), stop=(c == DCH - 1))
                    nc.tensor.matmul(p1, lhsT=WK[:, c, P:2 * P], rhs=HT[:, c, ssl],
                                     start=(c == 0), stop=(c == DCH - 1))
                rope_pair(p0, p1, KT[:, 0, ssl], KT[:, 1, ssl])
                # V directly in [s, dv] layout (lhsT = hT chunk slice)
                for si in range(SBLK // P):
                    sv = pp.tile([P, SBLK], F32, name="pp")[:, 0:HD]
                    soff = sb * SBLK + si * P
                    for c in range(DCH):
                        nc.tensor.matmul(sv, lhsT=HT[:, c, soff:soff + P],
                                         rhs=WV[:, c, :],
                                         start=(c == 0), stop=(c == DCH - 1))
                    nc.scalar.copy(VN[:, sb * (SBLK // P) + si, :], sv)

        # ---- late persistent: o_proj weights + normalized outT ----------
        patt = ctx.enter_context(tc.tile_pool(name="patt", bufs=1))
        WO = patt.tile([P, NQCH, D], BF16, name="WO")
        nc.sync.dma_start(out=WO, in_=wo.rearrange("(c p) n -> p c n", p=P))
        OUTN = patt.tile([P, NQCH, S], BF16, name="OUTN")

        # ---- phase C: attention + interleaved o_proj --------------------
        with tc.tile_pool(name="pexp", bufs=10) as pexp, \
             tc.tile_pool(name="pacc", bufs=4) as pacc, \
             tc.tile_pool(name="pou", bufs=8) as pou, \
             tc.tile_pool(name="pmisc", bufs=6) as pmisc, \
             tc.tile_pool(name="pmask", bufs=2) as pmask, \
             tc.tile_pool(name="pfin", bufs=3) as pfin, \
             tc.tile_pool(name="ps_s", bufs=4, space="PSUM") as ps_s, \
             tc.tile_pool(name="ps_o", bufs=2, space="PSUM") as ps_o, \
             tc.tile_pool(name="ps_f", bufs=2, space="PSUM") as ps_f:

            def emit_norm(pend):
                ou, acc, h, qb = pend
                qsl = slice(qb * SBLK, (qb + 1) * SBLK)
                pssum = ps_s.tile([P, SBLK], F32, name="ps")
                nc.tensor.matmul(pssum[0:1, :], lhsT=ONEC, rhs=acc)
                rsb = pmisc.tile([1, SBLK], F32, name="rsb")
                rsbb = pmisc.tile([1, SBLK], BF16, name="rsbb")
                with nc.allow_low_precision("approx reciprocal of softmax sums"):
                    nc.vector.reciprocal_approx_fast(out=rsb, in_=pssum[0:1, :])
                    nc.vector.tensor_copy(rsbb, rsb)
                psb = ps_f.tile([P, SBLK], F32, name="pf")
                nc.tensor.matmul(psb, lhsT=ONERB, rhs=rsbb)
                rbc = pmisc.tile([P, SBLK], F32R, name="rbc")
                nc.scalar.copy(rbc, psb)
                for dvc in range(2):
                    nc.vector.tensor_mul(OUTN[:, 2 * h + dvc, qsl], ou[dvc], rbc)

            def emit_oproj_quarter(qb):
                for st in range(4 * qb, 4 * qb + 4):
                    stsl = slice(st * P, (st + 1) * P)
                    for nb in range(NSBLK):
                        psf = ps_f.tile([P, SBLK], F32, name="pf")
                        for dvc in range(NQCH):
                            nc.tensor.matmul(psf, lhsT=OUTN[:, dvc, stsl],
                                             rhs=WO[:, dvc, nb * SBLK:(nb + 1) * SBLK],
                                             start=(dvc == 0), stop=(dvc == NQCH - 1))
                        # drain as bf16 (halves output DMA); alternate ACT/DVE
                        # so neither queue eats the full copy load (exp on ACT
                        # is latency-critical)
                        fsb = pfin.tile([P, SBLK], BF16, name="fsb")
                        if nb % 2 == 0:
                            nc.scalar.copy(fsb, psf)
                        else:
                            with nc.allow_low_precision("bf16 o_proj output"):
                                nc.vector.tensor_copy(fsb, psf)
                        nc.sync.dma_start(out=outp[stsl, nb * SBLK:(nb + 1) * SBLK],
                                          in_=fsb)

            pending = deque()
            for qb in range(NSBLK):
                qsl = slice(qb * SBLK, (qb + 1) * SBLK)
                klim = 4 * (qb + 1) if causal else NKC
                MT = None
                if not causal:
                    MT = pmask.tile([P, NKC, SBLK], BF16, name="mt")
                    nc.sync.dma_start(
                        out=MT,
                        in_=maskT.rearrange("(c p) n -> p c n", p=P)[:, :, qsl])
                for h in range(HEADS_PER_CORE):
                    pso = [ps_o.tile([P, SBLK], F32, name="po") for _ in range(2)]
                    acc = pacc.tile([P, SBLK], F32R, name="acc")
                    exs = {}

                    def emit_scores(kc):
                        pss = ps_s.tile([P, SBLK], F32, name="ps")
                        for c in range(NKCH):
                            nc.tensor.matmul(pss,
                                             lhsT=KT[:, c, kc * P:(kc + 1) * P],
                                             rhs=QT[:, 2 * h + c, qsl],
                                             start=(c == 0), stop=(c == NKCH - 1))
                        ex = pexp.tile([P, SBLK], BF16, name="ex")
                        nc.scalar.activation(ex, pss, EXP, scale=1.0 / 16.0)
                        if causal and kc >= 4 * qb:
                            delta = 128 * kc - 512 * qb
                            nc.vector.tensor_mul(ex, ex,
                                                 STAIR[:, 512 - delta:1024 - delta])
                        if not causal:
                            nc.vector.tensor_mul(ex, ex, MT[:, kc, :])
                        # denominator accumulation: all operands in SBUF, so it
                        # can run on the otherwise-idle gpsimd engine; split by
                        # head so neither engine's serial chain outruns the PE
                        eng = nc.vector if h == 0 else nc.gpsimd
                        if kc == 0:
                            eng.tensor_copy(acc, ex)
                        else:
                            eng.tensor_add(acc, acc, ex)
                        exs[kc] = ex

                    def emit_av(kc):
                        ex = exs.pop(kc)
                        for dvc in range(2):
                            nc.tensor.matmul(pso[dvc],
                                             lhsT=VN[:, kc, dvc * P:(dvc + 1) * P],
                                             rhs=ex, start=(kc == 0),
                                             stop=(kc == klim - 1))

                    LOOKAHEAD = 4
                    for kc in range(klim):
                        emit_scores(kc)
                        if kc >= LOOKAHEAD:
                            emit_av(kc - LOOKAHEAD)
                    for kc in range(max(0, klim - LOOKAHEAD), klim):
                        emit_av(kc)
                    ou = [pou.tile([P, SBLK], BF16, name="ou") for _ in range(2)]
                    for dvc in range(2):
                        nc.vector.tensor_copy(ou[dvc], pso[dvc])
                    pending.append((ou, acc, h, qb))
                    if len(pending) > 2:
                        p = pending.popleft()
                        emit_norm(p)
                        if p[2] == 1:
                            emit_oproj_quarter(p[3])
            while pending:
                p = pending.popleft()
                emit_norm(p)
                if p[2] == 1:
                    emit_oproj_quarter(p[3])

    nc.finalize()
    return nc


def _get_nc(causal: bool):
    key = bool(causal)
    if key not in _BUILD_CACHE:
        _BUILD_CACHE[key] = _build(causal)
    return _BUILD_CACHE[key]


def _rope_tables(position_ids_b):
    # cosT/sinT: [HD, S] fp32, transposed layout for the [d, s] dataflow
    pos = np.asarray(position_ids_b, dtype=np.float64)
    inv = 1.0 / (ROPE_BASE ** (np.arange(0, HD, 2, dtype=np.float64) / HD))
    f = pos[:, None] * inv[None, :]            # [S, HD/2]
    emb = np.concatenate([f, f], axis=1)       # [S, HD]
    cosT = np.ascontiguousarray(np.cos(emb).T.astype(np.float32))
    sinT = np.ascontiguousarray(np.sin(emb).T.astype(np.float32))
    return cosT, sinT


def _is_causal(attention_mask):
    m = np.asarray(attention_mask)
    if m.shape != (B, 1, S, S):
        return False
    tri = np.tril(np.ones((S, S), dtype=bool))
    canon = np.where(tri, np.float32(0.0), np.float32(-1e9))
    return all(np.array_equal(m[b, 0], canon) for b in range(B))


_ONES_NP = np.ones((P, P), dtype=np.float32)
_ONES_BF = np.ones((P, P), dtype=ml_dtypes.bfloat16)


def _stair():
    # multiplicative staircase: stair[p, j] = 1 if (j - 512) >= p else 0
    j = np.arange(2 * SBLK)[None, :] - SBLK
    p = np.arange(P)[:, None]
    return np.where(j >= p, 1.0, 0.0).astype(ml_dtypes.bfloat16)


def _bf(x):
    return np.ascontiguousarray(np.asarray(x, dtype=np.float32).astype(ml_dtypes.bfloat16))


def kernel(hidden_state, attention_mask, position_ids, Wq, Wk, Wv, Wo,
           _trace=False, _tmpdir=None):
    global LAST_EXEC_TIME_NS
    hidden_state = np.asarray(hidden_state, dtype=np.float32)

    causal = _is_causal(attention_mask)
    nc = _get_nc(causal)

    stair = _stair() if causal else None
    wk_bf = _bf(Wk)
    wv_bf = _bf(Wv)
    per_batch = {}
    for b in range(B):
        hTb = _bf(hidden_state[b].T)                           # [D, S] bf16
        cosT, sinT = _rope_tables(position_ids[b])
        mb = None
        if not causal:
            mb = np.ascontiguousarray(
                np.exp(np.asarray(attention_mask, dtype=np.float64)[b, 0].T)
                .astype(ml_dtypes.bfloat16))
        per_batch[b] = (hTb, cosT, sinT, mb)

    in_maps = []
    for core in range(8):
        b = core // 4
        hp = core % 4
        hTb, cosT, sinT, mb = per_batch[b]
        im = {
            "hT": hTb,
            "ones": _ONES_NP,
            "onesb": _ONES_BF,
            "wq": _bf(Wq[:, hp * DQ:(hp + 1) * DQ]),
            "wk": wk_bf,
            "wv": wv_bf,
            "wo": _bf(Wo[hp * DQ:(hp + 1) * DQ, :]),
            "cosT": cosT,
            "sinT": sinT,
        }
        if causal:
            im["stair"] = stair
        else:
            im["emaskT"] = mb
        in_maps.append(im)

    res = run_bass_kernel_spmd(nc, in_maps, core_ids=list(range(8)),
                               trace=_trace, tmpdir=_tmpdir)
    LAST_EXEC_TIME_NS = res.exec_time_ns

    out = np.empty((B, S, D), dtype=np.float32)
    for b in range(B):
        acc = res.results[4 * b]["out_partial"].astype(np.float32).copy()
        for hp in range(1, 4):
            acc += res.results[4 * b + hp]["out_partial"]
        out[b] = acc
    return out


# revision 22
# speedup vs baseline: 1.1104x; 1.0060x over previous
"""GemmaAttention (B=2, S=2048, D=2048, H=8, KV=1, HD=256) on 8 trn2 NeuronCores.

Sharding: DP=2 over batch x TP=4 over head-pairs. Core c handles batch c//4 and
heads {2*(c%4), 2*(c%4)+1}. Each core computes its partial o_proj output
(row-parallel Wo); the host sums the 4 partials per batch (the all-reduce is
folded into the host-side unshard).

All PE matmuls run in bf16 (error budget allows it): LDWEIGHTS is 1 cycle/row
instead of fp32r's ~4, and PE power/p-state behaves better. PSUM tiles are
always a full bank, so the kernel is structured around at most 8 live psum
tiles with single-group accumulation chains to keep the PE gap-free (the PE
only reaches its 2.4 GHz p-state after ~3us of continuous execution).

Dataflow per core:
  phase A (projections, full hT resident in SBUF as bf16):
    per s-block of 512 and output pair: psum <- sum_c W[:,c,pair].T @ hT[c,blk]
    RoPE applied in the psum->SBUF drain on DVE, output bf16 (QT/KT).
    V computed directly in [s, dv] layout: psum <- hT_chunk.T @ Wv, drained on
    the scalar engine to bf16 (VN).
  phase C (attention + interleaved o_proj):
    scoresT[k,q] = KT_chunk.T @ QT per head, exp on ACT (scale 1/16 folded),
    causal staircase as a bf16 multiplicative mask, denominators accumulated
    on DVE; outT[dv,q] += V_chunk.T @ expT; normalize with
    reciprocal_approx_fast; out_partial = outTn_chunk.T @ Wo -> DRAM.
"""

import numpy as np
import ml_dtypes

import concourse.bass as bass
import concourse.tile as tile
import concourse.mybir as mybir
from concourse import bacc
from concourse.bass_utils import run_bass_kernel_spmd
from concourse._compat import with_exitstack  # noqa: F401

P = 128
B, S, D = 2, 2048, 2048
H, KV, HD = 8, 1, 256
ROPE_BASE = 10000.0

HEADS_PER_CORE = 2
DQ = HEADS_PER_CORE * HD          # 512 q-dims per core
DCH = D // P                      # 16 contraction chunks
SBLK = 512                        # s-tile for projection rhs / q-tile
NSBLK = S // SBLK                 # 4
NKC = S // P                      # 16 key chunks
NQCH = DQ // P                    # 4 QT partition chunks
NKCH = HD // P                    # 2 KT partition chunks

F32 = mybir.dt.float32
F32R = mybir.dt.float32r
BF16 = mybir.dt.bfloat16
EXP = mybir.ActivationFunctionType.Exp

# exec time of the last traced run (set by run_spmd when tracing)
LAST_EXEC_TIME_NS = None

_BUILD_CACHE = {}


def _build(causal: bool):
    nc = bacc.Bacc()

    hT = nc.declare_dram_parameter("hT", [D, S], BF16, isOutput=False)
    wq = nc.declare_dram_parameter("wq", [D, DQ], BF16, isOutput=False)
    wk = nc.declare_dram_parameter("wk", [D, HD], BF16, isOutput=False)
    wv = nc.declare_dram_parameter("wv", [D, HD], BF16, isOutput=False)
    wo = nc.declare_dram_parameter("wo", [DQ, D], BF16, isOutput=False)
    cosT = nc.declare_dram_parameter("cosT", [HD, S], F32, isOutput=False)
    sinT = nc.declare_dram_parameter("sinT", [HD, S], F32, isOutput=False)
    ones = nc.declare_dram_parameter("ones", [P, P], F32R, isOutput=False)
    onesb = nc.declare_dram_parameter("onesb", [P, P], BF16, isOutput=False)
    if causal:
        stair = nc.declare_dram_parameter("stair", [P, 2 * SBLK], BF16, isOutput=False)
    else:
        maskT = nc.declare_dram_parameter("emaskT", [S, S], BF16, isOutput=False)
    outp = nc.declare_dram_parameter("out_partial", [S, D], BF16, isOutput=True)

    from contextlib import ExitStack
    from collections import deque
    with tile.TileContext(nc) as tc, ExitStack() as ctx:
        # persistent pools
        pq = ctx.enter_context(tc.tile_pool(name="pq", bufs=1))
        QT = pq.tile([P, NQCH, S], BF16, name="QT")
        KT = pq.tile([P, NKCH, S], BF16, name="KT")
        VN = pq.tile([P, NKC, HD], BF16, name="VN")
        ONES = pq.tile([P, P], F32R, name="ONES")
        ONESB = pq.tile([P, P], BF16, name="ONESB")
        ONEC = ONES[:, 0:1]
        ONERB = ONESB[0:1, :]
        if causal:
            STAIR = pq.tile([P, 2 * SBLK], BF16, name="STAIR")

        # ---- phase A: projections + RoPE --------------------------------
        with tc.tile_pool(name="pht", bufs=1) as pht, \
             tc.tile_pool(name="pw", bufs=1) as pw, \
             tc.tile_pool(name="pcs", bufs=1) as pcs, \
             tc.tile_pool(name="ptmp", bufs=8) as ptmp, \
             tc.tile_pool(name="pjp", bufs=8, space="PSUM") as pp:
            HT = pht.tile([P, DCH, S], BF16, name="HT")
            WQ = pw.tile([P, DCH, DQ], BF16, name="WQ")
            WK = pw.tile([P, DCH, HD], BF16, name="WK")
            WV = pw.tile([P, DCH, HD], BF16, name="WV")
            COS = pcs.tile([P, NKCH, S], F32, name="COS")
            SIN = pcs.tile([P, NKCH, S], F32, name="SIN")

            # DMA order matches consumption. Multi-chunk 3D transfers keep the
            # sync queue from becoming issue-bound (~620ns per descriptor).
            nc.sync.dma_start(out=ONES, in_=ones[:, :])
            nc.sync.dma_start(out=ONESB, in_=onesb[:, :])
            if causal:
                nc.sync.dma_start(out=STAIR, in_=stair[:, :])

            def chunked3d(out_tile, dram, c0, c1, csl):
                # out_tile[:, c0:c1, csl] <- dram rows [c0*P, c1*P), cols csl
                nc.sync.dma_start(
                    out=out_tile[:, c0:c1, csl],
                    in_=dram.rearrange("(c p) n -> p c n", p=P)[:, c0:c1, csl])

            full = slice(0, None)
            # sb0: WQ and hT slabs interleaved, small first slab for fast start
            for c0, c1 in ((0, 2), (2, 6), (6, 11), (11, 16)):
                chunked3d(WQ, wq, c0, c1, full)
                chunked3d(HT, hT, c0, c1, slice(0, SBLK))
            chunked3d(WK, wk, 0, DCH, full)
            chunked3d(WV, wv, 0, DCH, full)
            chunked3d(COS, cosT, 0, NKCH, full)
            chunked3d(SIN, sinT, 0, NKCH, full)
            for sb in range(1, NSBLK):
                chunked3d(HT, hT, 0, DCH, slice(sb * SBLK, (sb + 1) * SBLK))

            def rope_pair(p0, p1, out0, out1):
                # out0 = p0*cos0 - p1*sin0 ; out1 = p1*cos1 + p0*sin1
                c0 = COS[:, 0, ssl]; c1 = COS[:, 1, ssl]
                s0 = SIN[:, 0, ssl]; s1 = SIN[:, 1, ssl]
                t1 = ptmp.tile([P, SBLK], F32, name="t")
                t2 = ptmp.tile([P, SBLK], F32, name="t")
                nc.vector.tensor_mul(t1, p0, c0)
                nc.vector.tensor_mul(t2, p1, s0)
                nc.vector.tensor_sub(out0, t1, t2)
                t3 = ptmp.tile([P, SBLK], F32, name="t")
                t4 = ptmp.tile([P, SBLK], F32, name="t")
                nc.vector.tensor_mul(t3, p1, c1)
                nc.vector.tensor_mul(t4, p0, s1)
                nc.vector.tensor_add(out1, t3, t4)

            for sb in range(NSBLK):
                ssl = slice(sb * SBLK, (sb + 1) * SBLK)
                # Q pairs (per head), then K pair: 16-matmul psum groups
                for h in range(HEADS_PER_CORE):
                    p0 = pp.tile([P, SBLK], F32, name="pp")
                    p1 = pp.tile([P, SBLK], F32, name="pp")
                    for c in range(DCH):
                        nc.tensor.matmul(p0, lhsT=WQ[:, c, 2 * h * P:(2 * h + 1) * P],
                                         rhs=HT[:, c, ssl],
                                         start=(c == 0), stop=(c == DCH - 1))
                        nc.tensor.matmul(p1, lhsT=WQ[:, c, (2 * h + 1) * P:(2 * h + 2) * P],
                                         rhs=HT[:, c, ssl],
                                         start=(c == 0), stop=(c == DCH - 1))
                    rope_pair(p0, p1, QT[:, 2 * h, ssl], QT[:, 2 * h + 1, ssl])
                p0 = pp.tile([P, SBLK], F32, name="pp")
                p1 = pp.tile([P, SBLK], F32, name="pp")
                for c in range(DCH):
                    nc.tensor.matmul(p0, lhsT=WK[:, c, 0:P], rhs=HT[:, c, ssl],
                                     start=(c == 0), stop=(c == DCH - 1))
                    nc.tensor.matmul(p1, lhsT=WK[:, c, P:2 * P], rhs=HT[:, c, ssl],
                                     start=(c == 0), stop=(c == DCH - 1))
                rope_pair(p0, p1, KT[:, 0, ssl], KT[:, 1, ssl])
                # V directly in [s, dv] layout (lhsT = hT chunk slice)
                for si in range(SBLK // P):
                    sv = pp.tile([P, SBLK], F32, name="pp")[:, 0:HD]
                    soff = sb * SBLK + si * P
                    for c in range(DCH):
                        nc.tensor.matmul(sv, lhsT=HT[:, c, soff:soff + P],
                                         rhs=WV[:, c, :],
                                         start=(c == 0), stop=(c == DCH - 1))
                    nc.scalar.copy(VN[:, sb * (SBLK // P) + si, :], sv)

        # ---- late persistent: o_proj weights + normalized outT ----------
        patt = ctx.enter_context(tc.tile_pool(name="patt", bufs=1))
        WO = patt.tile([P, NQCH, D], BF16, name="WO")
        nc.sync.dma_start(out=WO, in_=wo.rearrange("(c p) n -> p c n", p=P))
        OUTN = patt.tile([P, NQCH, S], BF16, name="OUTN")

        # ---- phase C: attention + interleaved o_proj --------------------
        with tc.tile_pool(name="pexp", bufs=10) as pexp, \
             tc.tile_pool(name="pacc", bufs=12) as pacc, \
             tc.tile_pool(name="pou", bufs=8) as pou, \
             tc.tile_pool(name="pmisc", bufs=6) as pmisc, \
             tc.tile_pool(name="pmask", bufs=2) as pmask, \
             tc.tile_pool(name="pfin", bufs=3) as pfin, \
             tc.tile_pool(name="ps_s", bufs=4, space="PSUM") as ps_s, \
             tc.tile_pool(name="ps_o", bufs=2, space="PSUM") as ps_o, \
             tc.tile_pool(name="ps_f", bufs=2, space="PSUM") as ps_f:

            def emit_norm(pend):
                ou, acc, h, qb = pend
                qsl = slice(qb * SBLK, (qb + 1) * SBLK)
                pssum = ps_s.tile([P, SBLK], F32, name="ps")
                nc.tensor.matmul(pssum[0:1, :], lhsT=ONEC, rhs=acc)
                rsb = pmisc.tile([1, SBLK], F32, name="rsb")
                rsbb = pmisc.tile([1, SBLK], BF16, name="rsbb")
                with nc.allow_low_precision("approx reciprocal of softmax sums"):
                    nc.vector.reciprocal_approx_fast(out=rsb, in_=pssum[0:1, :])
                    nc.vector.tensor_copy(rsbb, rsb)
                psb = ps_f.tile([P, SBLK], F32, name="pf")
                nc.tensor.matmul(psb, lhsT=ONERB, rhs=rsbb)
                rbc = pmisc.tile([P, SBLK], F32R, name="rbc")
                nc.scalar.copy(rbc, psb)
                for dvc in range(2):
                    nc.vector.tensor_mul(OUTN[:, 2 * h + dvc, qsl], ou[dvc], rbc)

            def emit_oproj_quarter(qb):
                for st in range(4 * qb, 4 * qb + 4):
                    stsl = slice(st * P, (st + 1) * P)
                    for nb in range(NSBLK):
                        psf = ps_f.tile([P, SBLK], F32, name="pf")
                        for dvc in range(NQCH):
                            nc.tensor.matmul(psf, lhsT=OUTN[:, dvc, stsl],
                                             rhs=WO[:, dvc, nb * SBLK:(nb + 1) * SBLK],
                                             start=(dvc == 0), stop=(dvc == NQCH - 1))
                        # drain as bf16 (halves output DMA); alternate ACT/DVE
                        # so neither queue eats the full copy load (exp on ACT
                        # is latency-critical)
                        fsb = pfin.tile([P, SBLK], BF16, name="fsb")
                        if nb % 2 == 0:
                            nc.scalar.copy(fsb, psf)
                        else:
                            with nc.allow_low_precision("bf16 o_proj output"):
                                nc.vector.tensor_copy(fsb, psf)
                        nc.sync.dma_start(out=outp[stsl, nb * SBLK:(nb + 1) * SBLK],
                                          in_=fsb)

            pending = deque()
            for qb in range(NSBLK):
                qsl = slice(qb * SBLK, (qb + 1) * SBLK)
                klim = 4 * (qb + 1) if causal else NKC
                MT = None
                if not causal:
                    MT = pmask.tile([P, NKC, SBLK], BF16, name="mt")
                    nc.sync.dma_start(
                        out=MT,
                        in_=maskT.rearrange("(c p) n -> p c n", p=P)[:, :, qsl])
                for h in range(HEADS_PER_CORE):
                    pso = [ps_o.tile([P, SBLK], F32, name="po") for _ in range(2)]
                    exs = {}
                    # denominator accumulation as a pairwise tree on the
                    # otherwise-idle engines (DVE for h0, gpsimd for h1): the
                    # serial chain after the last ex is ~log2 deep instead of
                    # klim deep, so the norm matmul never stalls the PE
                    eng = nc.vector if h == 0 else nc.gpsimd
                    parts = []

                    def acc_push(node):
                        rank = 0
                        while parts and parts[-1][0] == rank:
                            _, prev = parts.pop()
                            t = pacc.tile([P, SBLK], F32R, name="acc")
                            eng.tensor_add(t, prev, node)
                            node = t
                            rank += 1
                        parts.append((rank, node))

                    def acc_flush():
                        _, node = parts.pop()
                        while parts:
                            _, prev = parts.pop()
                            t = pacc.tile([P, SBLK], F32R, name="acc")
                            eng.tensor_add(t, prev, node)
                            node = t
                        return node

                    def emit_scores(kc):
                        pss = ps_s.tile([P, SBLK], F32, name="ps")
                        for c in range(NKCH):
                            nc.tensor.matmul(pss,
                                             lhsT=KT[:, c, kc * P:(kc + 1) * P],
                                             rhs=QT[:, 2 * h + c, qsl],
                                             start=(c == 0), stop=(c == NKCH - 1))
                        ex = pexp.tile([P, SBLK], BF16, name="ex")
                        nc.scalar.activation(ex, pss, EXP, scale=1.0 / 16.0)
                        if causal and kc >= 4 * qb:
                            delta = 128 * kc - 512 * qb
                            nc.vector.tensor_mul(ex, ex,
                                                 STAIR[:, 512 - delta:1024 - delta])
                        if not causal:
                            nc.vector.tensor_mul(ex, ex, MT[:, kc, :])
                        acc_push(ex)
                        exs[kc] = ex

                    def emit_av(kc):
                        ex = exs.pop(kc)
                        for dvc in range(2):
                            nc.tensor.matmul(pso[dvc],
                                             lhsT=VN[:, kc, dvc * P:(dvc + 1) * P],
                                             rhs=ex, start=(kc == 0),
                                             stop=(kc == klim - 1))

                    LOOKAHEAD = 4
                    for kc in range(klim):
                        emit_scores(kc)
                        if kc >= LOOKAHEAD:
                            emit_av(kc - LOOKAHEAD)
                    for kc in range(max(0, klim - LOOKAHEAD), klim):
                        emit_av(kc)
                    acc = acc_flush()
                    ou = [pou.tile([P, SBLK], BF16, name="ou") for _ in range(2)]
                    for dvc in range(2):
                        nc.vector.tensor_copy(ou[dvc], pso[dvc])
                    pending.append((ou, acc, h, qb))
                    if len(pending) > 2:
                        p = pending.popleft()
                        emit_norm(p)
                        if p[2] == 1:
                            emit_oproj_quarter(p[3])
            while pending:
                p = pending.popleft()
                emit_norm(p)
                if p[2] == 1:
                    emit_oproj_quarter(p[3])

    nc.finalize()
    return nc


def _get_nc(causal: bool):
    key = bool(causal)
    if key not in _BUILD_CACHE:
        _BUILD_CACHE[key] = _build(causal)
    return _BUILD_CACHE[key]


def _rope_tables(position_ids_b):
    # cosT/sinT: [HD, S] fp32, transposed layout for the [d, s] dataflow
    pos = np.asarray(position_ids_b, dtype=np.float64)
    inv = 1.0 / (ROPE_BASE ** (np.arange(0, HD, 2, dtype=np.float64) / HD))
    f = pos[:, None] * inv[None, :]            # [S, HD/2]
    emb = np.concatenate([f, f], axis=1)       # [S, HD]
    cosT = np.ascontiguousarray(np.cos(emb).T.astype(np.float32))
    sinT = np.ascontiguousarray(np.sin(emb).T.astype(np.float32))
    return cosT, sinT


def _is_causal(attention_mask):
    m = np.asarray(attention_mask)
    if m.shape != (B, 1, S, S):
        return False
    tri = np.tril(np.ones((S, S), dtype=bool))
    canon = np.where(tri, np.float32(0.0), np.float32(-1e9))
    return all(np.array_equal(m[b, 0], canon) for b in range(B))


_ONES_NP = np.ones((P, P), dtype=np.float32)
_ONES_BF = np.ones((P, P), dtype=ml_dtypes.bfloat16)


def _stair():
    # multiplicative staircase: stair[p, j] = 1 if (j - 512) >= p else 0
    j = np.arange(2 * SBLK)[None, :] - SBLK
    p = np.arange(P)[:, None]
    return np.where(j >= p, 1.0, 0.0).astype(ml_dtypes.bfloat16)


def _bf(x):
    return np.ascontiguousarray(np.asarray(x, dtype=np.float32).astype(ml_dtypes.bfloat16))


def kernel(hidden_state, attention_mask, position_ids, Wq, Wk, Wv, Wo,
           _trace=False, _tmpdir=None):
    global LAST_EXEC_TIME_NS
    hidden_state = np.asarray(hidden_state, dtype=np.float32)

    causal = _is_causal(attention_mask)
    nc = _get_nc(causal)

    stair = _stair() if causal else None
    wk_bf = _bf(Wk)
    wv_bf = _bf(Wv)
    per_batch = {}
    for b in range(B):
        hTb = _bf(hidden_state[b].T)                           # [D, S] bf16
        cosT, sinT = _rope_tables(position_ids[b])
        mb = None
        if not causal:
            mb = np.ascontiguousarray(
                np.exp(np.asarray(attention_mask, dtype=np.float64)[b, 0].T)
                .astype(ml_dtypes.bfloat16))
        per_batch[b] = (hTb, cosT, sinT, mb)

    in_maps = []
    for core in range(8):
        b = core // 4
        hp = core % 4
        hTb, cosT, sinT, mb = per_batch[b]
        im = {
            "hT": hTb,
            "ones": _ONES_NP,
            "onesb": _ONES_BF,
            "wq": _bf(Wq[:, hp * DQ:(hp + 1) * DQ]),
            "wk": wk_bf,
            "wv": wv_bf,
            "wo": _bf(Wo[hp * DQ:(hp + 1) * DQ, :]),
            "cosT": cosT,
            "sinT": sinT,
        }
        if causal:
            im["stair"] = stair
        else:
            im["emaskT"] = mb
        in_maps.append(im)

    res = run_bass_kernel_spmd(nc, in_maps, core_ids=list(range(8)),
                               trace=_trace, tmpdir=_tmpdir)
    LAST_EXEC_TIME_NS = res.exec_time_ns

    out = np.empty((B, S, D), dtype=np.float32)
    for b in range(B):
        acc = res.results[4 * b]["out_partial"].astype(np.float32).copy()
        for hp in range(1, 4):
            acc += res.results[4 * b + hp]["out_partial"]
        out[b] = acc
    return out
